# revision 26
# baseline (speedup 1.0000x reference)
"""Grouped GEMM (MoE routing) Trainium2 kernel.

Strategy: tensor-parallel shard of the output N dim across 8 NeuronCores.
Every core sees all T=8192 tokens and a 512-wide slice of every expert's
weights, so per-core work is identical regardless of segment sizes and a
single SPMD program (with the segment boundaries baked in as compile-time
constants) runs on all 8 cores.

Per core:  out_t[n, t] = sum_k w_t[e(t), k, n] * a_t[k, t]

Inputs and outputs are cast to bf16 on the host (rel err ~3e-3, far
under the 2e-2 gate), cutting HBM traffic to ~105MB/core.  All DMAs
are laid out so each SBUF partition line is one contiguous HBM run.
a-block loads ride the sync HWDGE queue; weight loads + output stores
ride the scalar HWDGE queue, the stores doubling as in-order pacing
gates for the next expert's weight chunks, which are spread across the
current run's blocks.  Experts are processed in descending
segment-length order so every expert switch is covered by a long
compute run.

Startup shaping (two cores share one 716GB/s HBM stack, and the
sequencers' own instruction fetches ride the same DMA fabric, so the
8-core startup flood is the main source of per-core variance): only
w0 + a0 + a1 (~11.8MB) may be queued before block0 completes; block2's
a-load is emitted behind block0's store gate on the scalar ring.  A
short burst of dummy matmuls on scratch SBUF keeps the PE busy until
the first chunks land and opens the HAM clock-gate (1.2GHz -> full
clock ~10.7us after first PE activity).

Matmul mapping: stationary lhsT = w tile [k=128, n=128], moving rhs =
a tile [k=128, tok<=512] in bf16, PSUM out [n=128, tok<=512] fp32,
accumulated over the 32 k-chunks (ko-outer: nb-outer chains cost a
PSUM stop->start turnaround per chain, +10us/kernel).  Compute
floor/core = T*K*NS/(128*128) cycles = 437us @2.4GHz; measured
462-477us cool, ~555us when the part thermally throttles to ~2.0GHz
(then the kernel is gapless/PE-bound and the floor is ~530us).
"""

import numpy as np
import ml_dtypes

import concourse.bacc as bacc
import concourse.bass as bass
import concourse.mybir as mybir
import concourse.tile as tile
from concourse.bass_utils import run_bass_kernel_spmd

NC = 8          # NeuronCores
P = 128         # partitions
TB = 512        # max token block (PSUM bank = 512 fp32)

BF16 = ml_dtypes.bfloat16

LAST_RESULT = {}


def _token_blocks(seg_starts, seg_ends):
    """Split each segment into balanced pieces of <=512 tokens, keeping all
    but at most one piece even-length (odd pieces misalign the bf16 k-chunk
    slices and cost ~0.15us/block).

    The first (longest) segment is instead split [512, rem, 512, ...]: a
    maximal block0 gives the startup DMA flood the most time per byte, and
    a minimal block1 shrinks a1 -- the load whose deadline (block1's start)
    is what the losing core of each HBM-stack pair misses."""
    blocks = []  # (tstart, tlen, active_expert_idx)
    for widx, (s, t) in enumerate(zip(seg_starts, seg_ends)):
        ln = t - s
        npieces = max(1, -(-ln // TB))
        if widx == 0 and npieces > 1:
            rem = ln - TB * (npieces - 1)
            sizes = [TB, rem] + [TB] * (npieces - 2)
        else:
            base2 = (ln // npieces) & ~1
            sizes = [base2] * npieces
            rem = ln - base2 * npieces
            for i in range(rem // 2):
                sizes[i] += 2
            if rem % 2:
                sizes[-1] += 1
        p = s
        for L in sizes:
            if L > 0:
                blocks.append((p, L, widx))
                p += L
    return blocks


WCH = 8         # ko per weight/a DMA chunk (4 chunks of 8KB+/partition)


def _build_program(T, K, NS, EA, blocks):
    f32 = mybir.dt.float32
    bf16 = mybir.dt.bfloat16
    KO = K // P
    NB = NS // P
    NCH = KO // WCH

    # per-ko stride padded to even so every k-chunk slice stays 4B-aligned
    CTA = sum(KO * (L + L % 2) for (_, L, _) in blocks)
    CTO = sum(NB * L for (_, L, _) in blocks)

    # group consecutive same-expert blocks into runs
    runs = []
    for blk in blocks:
        if runs and runs[-1][0] == blk[2]:
            runs[-1][1].append(blk)
        else:
            runs.append((blk[2], [blk]))

    nc = bacc.Bacc(None, target_bir_lowering=False)
    ab = nc.declare_dram_parameter("ab", [P, CTA], bf16, isOutput=False)
    wb = nc.declare_dram_parameter("wb", [EA, P, KO, NS], bf16, isOutput=False)
    ot = nc.declare_dram_parameter("ot", [P, CTO], bf16, isOutput=True)

    with tile.TileContext(nc) as tc:
        with (
            tc.tile_pool(name="wpool", bufs=3) as wpool,
            tc.tile_pool(name="apool", bufs=3) as apool,
            tc.tile_pool(name="opool", bufs=2) as opool,
            tc.tile_pool(name="psum", bufs=2, space=bass.MemorySpace.PSUM) as psum_pool,
        ):
            def load_w_range(wt, widx, s, e):
                nc.scalar.dma_start(
                    out=wt[:, s:e, :],
                    in_=wb[widx, :, s:e, :])

            # ko ranges: uniform 4-ko chunks for the first block/weights so
            # delivery stays ahead of the MM stream all the way through
            # block 0 (coarser mid-block chunks stall at their boundary;
            # finer leading chunks start MMs too early and stall the same
            # way - both measured +5-6us).  Steady-state blocks load in two
            # 16-ko halves (a single whole-block DMA measured +2us: its
            # delivery tail lands right when the block starts).
            FINE = [(4 * i, 4 * i + 4) for i in range(8)]
            # leading 2-ko chunks: halves the bytes the first matmul waits
            # for under the 8-core startup flood, and keeps the early chunk
            # cadence fine through ko 0-7 where delivery jitter bites
            FINE0 = [(2 * i, 2 * i + 2) for i in range(4)] + FINE[2:]
            COARSE = [(0, 16), (16, 32)]

            # PE pre-warm: dummy matmuls on scratch SBUF with no DMA deps.
            # The HAM clock-gate (1.2GHz -> full clock) opens ~10.7us after
            # the first sustained PE activity, regardless of short gaps;
            # the first real matmul is gated by its first DMA chunks
            # (~14.8us under the 8-core startup flood).  28 matmuls keep
            # the PE busy until just before the data lands -- longer
            # warmups push the first real matmul later for no clock
            # benefit, shorter ones leave the gate timer the same.
            warm_w = wpool.tile([P, P], bf16, tag="warm", name="warm_w")
            warm_a = apool.tile([P, 192], bf16, tag="warm", name="warm_a")
            nc.vector.memset(warm_w[:, :], 0)
            nc.vector.memset(warm_a[:, :], 0)
            warm_ps = psum_pool.tile([P, 1, 192], f32, tag="ps", name="warm_ps",
                                     padded_shape=[P, NB, TB])
            for _ in range(28):
                nc.tensor.matmul(warm_ps[:, 0, :], warm_w[:, :], warm_a[:, :],
                                 start=True, stop=True)

            # flatten (run, block) structure; precompute per-block offsets
            flat = []  # (ri, bi, nbk, L)
            for ri, (widx, rblocks) in enumerate(runs):
                for bi, (ts, L, _) in enumerate(rblocks):
                    flat.append((ri, bi, len(rblocks), L))
            offs_a = []
            offs_o = []
            oa = oo = 0
            for (_, _, _, L) in flat:
                offs_a.append(oa)
                offs_o.append(oo)
                oa += KO * (L + L % 2)
                oo += NB * L

            a_tiles = {}

            def emit_a_load(idx, eng, ranges):
                L = flat[idx][3]
                Lp = L + L % 2
                at = apool.tile([P, KO * Lp], bf16, tag="a", name="a_tile",
                                padded_shape=[P, KO * TB])
                for (s, e) in ranges:
                    eng.dma_start(
                        out=at[:, s * Lp:e * Lp],
                        in_=ab[:, offs_a[idx] + s * Lp:offs_a[idx] + e * Lp])
                a_tiles[idx] = at

            nblk = len(flat)
            w_next = wpool.tile([P, KO, NS], bf16, tag="w", name="w_tile")
            for (s, e) in FINE0:
                load_w_range(w_next, runs[0][0], s, e)
            a1_h0 = None
            w_tile = None
            for i, (ri, bi, nbk, L) in enumerate(flat):
                if bi == 0:
                    w_tile = w_next
                    if ri + 1 < len(runs):
                        w_next = wpool.tile([P, KO, NS], bf16, tag="w",
                                            name="w_tile")
                Lp = L + L % 2
                if i not in a_tiles:
                    emit_a_load(i, nc.sync, FINE0 if i == 0 else COARSE)
                elif i == 1 and a1_h0 is not None:
                    # a1's first half on the sync ring at block1's top
                    at1, Lp1 = a1_h0
                    (s, e) = COARSE[0]
                    nc.sync.dma_start(
                        out=at1[:, s * Lp1:e * Lp1],
                        in_=ab[:, offs_a[1] + s * Lp1:offs_a[1] + e * Lp1])
                a_tile = a_tiles.pop(i)
                off_o = offs_o[i]
                ptile = psum_pool.tile([P, NB, L], f32, tag="ps", name="ps",
                                       padded_shape=[P, NB, TB])
                for ko in range(KO):
                    for nb in range(NB):
                        nc.tensor.matmul(
                            ptile[:, nb, :],
                            w_tile[:, ko, nb * P:(nb + 1) * P],
                            a_tile[:, ko * Lp:ko * Lp + L],
                            start=(ko == 0),
                            stop=(ko == KO - 1),
                        )
                o_tile = opool.tile([P, NB * L], bf16, tag="o", name="o_tile",
                                    padded_shape=[P, NB * TB])
                for nb in range(NB):
                    nc.vector.tensor_copy(o_tile[:, nb * L:(nb + 1) * L],
                                          ptile[:, nb, :])
                if i == nblk - 1:
                    # split the final store across both rings: its ~128
                    # small descriptors per half drain in parallel, halving
                    # the post-last-matmul tail
                    half = (NB // 2) * L
                    nc.scalar.dma_start(out=ot[:, off_o:off_o + half],
                                        in_=o_tile[:, :half])
                    nc.sync.dma_start(out=ot[:, off_o + half:off_o + NB * L],
                                      in_=o_tile[:, half:NB * L])
                else:
                    nc.scalar.dma_start(out=ot[:, off_o:off_o + NB * L],
                                        in_=o_tile[:, :])
                if i == 0:
                    # a1's second half and block2's a-load ride the scalar
                    # ring *behind* block0's store: they dispatch once
                    # block0's casts are done (~45.5us) and land before
                    # block1's midpoint (~55us) / block2's start (~66us)
                    # need them.  This caps the pre-block0 queue at
                    # w0+a0+a1h0 (9.8MB): the sync ring then meets block1's
                    # start even at the losing core's arbitration share
                    # (the ~4.5us a1 gap on the slow cores), and more
                    # early bytes would starve the sequencers' instruction
                    # fetches (TENSOR_LOAD rides the same DMA fabric),
                    # stalling every core ~10us.
                    if nblk > 1:
                        L1 = flat[1][3]
                        Lp1 = L1 + L1 % 2
                        at1 = apool.tile([P, KO * Lp1], bf16, tag="a",
                                         name="a_tile",
                                         padded_shape=[P, KO * TB])
                        (s, e) = COARSE[1]
                        nc.scalar.dma_start(
                            out=at1[:, s * Lp1:e * Lp1],
                            in_=ab[:, offs_a[1] + s * Lp1:
                                    offs_a[1] + e * Lp1])
                        a_tiles[1] = at1
                        a1_h0 = (at1, Lp1)
                    if nblk > 3:
                        emit_a_load(2, nc.scalar, COARSE)
                # pace the next expert's weight chunks across this run's
                # blocks so the prefetch never bursts against the a-stream
                if ri + 1 < len(runs):
                    c0 = bi * NCH // nbk
                    c1 = (bi + 1) * NCH // nbk
                    for c in range(c0, c1):
                        load_w_range(w_next, runs[ri + 1][0],
                                     c * WCH, (c + 1) * WCH)
    nc.compile()
    return nc


def kernel(a, b, c, seg_indptr, weight_indices, batch_size, **_):
    T, K = a.shape
    E, N, K2 = b.shape
    assert K == K2
    NS = N // NC
    KO = K // P
    NB = NS // P

    seg = np.asarray(seg_indptr).astype(np.int64)
    widx_arr = np.asarray(weight_indices).astype(np.int64)
    segs = [(int(seg[e]), int(seg[e + 1]), int(widx_arr[e]))
            for e in range(int(batch_size)) if seg[e + 1] > seg[e]]
    # process longest segments first: every expert switch is then covered by
    # a long compute run, hiding the next weight load entirely
    segs.sort(key=lambda s: s[0] - s[1])
    seg_starts = [s for s, _, _ in segs]
    seg_ends = [t for _, t, _ in segs]
    experts = [w for _, _, w in segs]
    EA = len(segs)
    blocks = _token_blocks(seg_starts, seg_ends)

    # a -> [P, KO, T] bf16 (partition-major k layout), then pack blocks so
    # each block is a [P, KO*L] slab with 32KB-contiguous partition lines.
    a = np.ascontiguousarray(a, dtype=np.float32)
    at_full = a.T.reshape(KO, P, T).transpose(1, 0, 2).astype(BF16)  # [P,KO,T]
    CTA = sum(KO * (L + L % 2) for (_, L, _) in blocks)
    ab_np = np.zeros((P, CTA), dtype=BF16)
    off = 0
    for (ts, L, _) in blocks:
        Lp = L + L % 2
        tmp = np.zeros((P, KO, Lp), dtype=BF16)
        tmp[:, :, :L] = at_full[:, :, ts:ts + L]
        ab_np[:, off:off + KO * Lp] = tmp.reshape(P, KO * Lp)
        off += KO * Lp

    # weights: full [E_active, P, KO, N] bf16 once, slice per core.
    wt_full = np.empty((EA, P, KO, N), dtype=BF16)
    for ei, e in enumerate(experts):
        wt_full[ei] = b[e].T.reshape(KO, P, N).transpose(1, 0, 2)

    in_maps = []
    for j in range(NC):
        w = np.ascontiguousarray(wt_full[:, :, :, j * NS:(j + 1) * NS])
        in_maps.append({"ab": ab_np, "wb": w})

    nc = _build_program(T, K, NS, EA, blocks)

    import os
    trace = bool(int(os.environ.get("BASS_KERNEL_TRACE", "0")))
    res = run_bass_kernel_spmd(nc, in_maps, list(range(NC)), trace=trace)
    LAST_RESULT["exec_time_ns"] = res.exec_time_ns
    LAST_RESULT["results"] = res

    out_t = np.empty((N, T), dtype=np.float32)
    for j in range(NC):
        otj = np.asarray(res.results[j]["ot"]).astype(np.float32)  # [P, CTO]
        off = 0
        for (ts, L, _) in blocks:
            blk = otj[:, off:off + NB * L].reshape(P, NB, L)
            out_t[j * NS:(j + 1) * NS, ts:ts + L] = (
                blk.transpose(1, 0, 2).reshape(NS, L))
            off += NB * L
    return np.ascontiguousarray(out_t.T)

